# revision 4
# baseline (speedup 1.0000x reference)
"""Trainium2 Bass kernel for the MFA/MPPCA mixture log-likelihood problem.

Math: out[n,k] = PI[k] + logprob[n,k] with Sigma_k = A_k A_k^T + diag(D_k^2),
computed via Woodbury.  Everything involving only the small parameters
(MU, A, D, PI) is folded on the host into:

    out[n,k] = CONST[k] + x[n]·H[:,k] + (x[n]^2)·G[:,k] + sum_l (x[n]·Csc[:,k,l])^2

where (with iD = D^-2, B = iD*A, L = I + A^T B, iL = inv(L), R = chol(iL),
C0 = B R, e = R^T B^T MU):
    G   = -0.5 * iD^T                       (d, K)
    H   = (iD*MU)^T - C0 e                  (d, K)
    Csc = sqrt(0.5) * C0                    (d, K*l)
    CONST = PI - 0.5*(d log 2pi + logdet Sigma + MU^T iD MU) + 0.5 |e|^2

Device kernel (data-parallel over N on 8 cores, x / x^2 pre-transposed and
pre-tiled on host):
  PE:     x·[H|Csc] as fp8e4 DoubleRow matmuls (256-deep contraction),
          x^2·G as fp16 matmuls accumulating into the same psum H block.
  Scalar: squares factor groups 0:32 (psum_a) and copies the H+G psum block
          into the shared sq tile.
  Vector: squares factor groups 32:64 (psum_b), final group-of-6 reduce.
  GpSimd: pairwise pre-reduction (12 -> 6) of the sq tile.
  CONST is a static column of the sq tile, folded in by the reduce.
Output is fp16 on device, cast to fp32 on host.
"""
import math
import numpy as np
import ml_dtypes

N_TOTAL, K, D_FEAT, L_FAC = 131072, 64, 512, 10
N_CORES = 8
N_PER_CORE = N_TOTAL // N_CORES  # 16384

WALL_COLS = K + K * L_FAC  # 704 = [H (0:64) | Csc (64:704)]
NGA = 32                   # factor groups in psum_a -> psum_a = 64 + 320 = 384
NGB = K - NGA              # factor groups in psum_b -> 320


def host_prep(MU, A, D, PI):
    """Fold small-parameter math into matmul weights (float64 internally)."""
    MU64, A64, D64, PI64 = [np.asarray(v, np.float64) for v in (MU, A, D, PI)]
    Kc, d, l = A64.shape
    iD = D64 ** -2.0
    B = iD[..., None] * A64
    L = np.eye(l)[None] + np.einsum('kdl,kdm->klm', A64, B)
    sign, logdet_L = np.linalg.slogdet(L)
    log_det_Sigma = logdet_L - np.sum(np.log(iD), axis=1)
    iL = np.linalg.inv(L)
    R = np.linalg.cholesky(iL)                  # R @ R.T = iL
    C0 = np.einsum('kdl,klm->kdm', B, R)        # (K, d, l)
    bmu = np.einsum('kdl,kd->kl', B, MU64)
    e = np.einsum('klm,kl->km', R, bmu)         # (K, l)
    c1 = np.sum(iD * MU64 * MU64, axis=1)

    CONST = PI64 - 0.5 * (d * math.log(2.0 * math.pi) + log_det_Sigma + c1) \
        + 0.5 * np.sum(e * e, axis=1)
    G = (-0.5 * iD).T
    H = (iD * MU64 - np.einsum('kdm,km->kd', C0, e)).T
    Csc = (C0 * np.sqrt(0.5)).transpose(1, 0, 2).reshape(d, Kc * l)  # k-major

    wall = np.concatenate([H, Csc], axis=1).astype(ml_dtypes.float8_e4m3)
    g16 = G.astype(np.float16)                                      # (d, K)
    cfill = np.tile(CONST.astype(np.float16)[None, :], (128, 1))    # (128, K)
    return wall, g16, cfill


def _tile_xt(xt, dtype):
    """(d, n) -> (128, n_sub*4*128) so each partition's tile data is one
    contiguous run: arr[p, i, c, n] = xt[c*128+p, i*128+n]."""
    d, n = xt.shape
    n_sub = n // 128
    a = xt.reshape(4, 128, n_sub, 128)          # [c, p, i, n]
    a = a.transpose(1, 2, 0, 3)                 # [p, i, c, n]
    return np.ascontiguousarray(a.astype(dtype)).reshape(128, n_sub * 4 * 128)


def build_nc(n_per_core=N_PER_CORE):
    """Build and compile the Bass module for one core (SPMD across 8)."""
    import concourse.bacc as bacc
    import concourse.tile as tile
    import concourse.mybir as mybir

    f32 = mybir.dt.float32
    f16 = mybir.dt.float16
    f8 = mybir.dt.float8e4
    DR = mybir.MatmulPerfMode.DoubleRow
    n_sub = n_per_core // 128
    assert n_per_core % 128 == 0

    nc = bacc.Bacc("TRN2", target_bir_lowering=False, debug=False,
                   enable_asserts=False, num_devices=N_CORES)
    xt_dram = nc.dram_tensor("xt", (128, n_sub * 4 * 128), f8, kind="ExternalInput")
    x2t_dram = nc.dram_tensor("x2t", (128, n_sub * 4 * 128), f16, kind="ExternalInput")
    wall_dram = nc.dram_tensor("wall", (D_FEAT, WALL_COLS), f8, kind="ExternalInput")
    g_dram = nc.dram_tensor("g16", (D_FEAT, K), f16, kind="ExternalInput")
    c_dram = nc.dram_tensor("cfill", (128, K), f16, kind="ExternalInput")
    out_dram = nc.dram_tensor("out", (n_per_core, K), f16, kind="ExternalOutput")

    xt_v = xt_dram.ap().rearrange("p (i c n) -> p i c n", c=4, n=128)
    x2t_v = x2t_dram.ap().rearrange("p (i c n) -> p i c n", c=4, n=128)
    wall_v = wall_dram.ap().rearrange("(c p) m -> p c m", p=128)   # [128, 4, 704]
    g_v = g_dram.ap().rearrange("(c p) m -> p c m", p=128)         # [128, 4, 64]

    wca = K + NGA * L_FAC  # 384

    with tile.TileContext(nc) as tc, nc.allow_low_precision("fp16 within rel tolerance"):
        with (
            tc.tile_pool(name="wpool", bufs=1) as wpool,
            tc.tile_pool(name="xpool", bufs=4) as xpool,
            tc.tile_pool(name="rpool", bufs=3) as rpool,
            tc.tile_pool(name="opool", bufs=4) as opool,
            tc.tile_pool(name="ppool", bufs=3, space="PSUM") as ppool,
        ):
            wall_sb = wpool.tile([128, 4, WALL_COLS], f8)
            nc.sync.dma_start(out=wall_sb[:], in_=wall_v[:])
            g_sb = wpool.tile([128, 4, K], f16)
            nc.sync.dma_start(out=g_sb[:], in_=g_v[:])

            # shared square tile, manually double buffered:
            # [:, b, k, 0:10] squares | [:, b, k, 10] CONST | [:, b, k, 11] H+G
            sq = wpool.tile([128, 2, K, 12], f16)
            nc.sync.dma_start(out=sq[:, 0, :, 10], in_=c_dram.ap())
            nc.sync.dma_start(out=sq[:, 1, :, 10], in_=c_dram.ap())

            for i in range(n_sub):
                xt_sb = xpool.tile([128, 4, 128], f8, tag="xt")
                nc.sync.dma_start(out=xt_sb[:], in_=xt_v[:, i])
                x2t_sb = xpool.tile([128, 4, 128], f16, tag="x2t")
                nc.sync.dma_start(out=x2t_sb[:], in_=x2t_v[:, i])

                psum_a = ppool.tile([128, K + NGA * L_FAC], f32, tag="pa")
                psum_b = ppool.tile([128, NGB * L_FAC], f32, tag="pb")

                # fp8 DoubleRow: 256-deep contraction per matmul, 2 chunk-pairs;
                # x^2·G (fp16) sandwiched so the final DR matmuls close groups
                nc.tensor.matmul(psum_a[:], xt_sb[:, 0:2, :],
                                 wall_sb[:, 0:2, 0:wca],
                                 start=True, stop=False, perf_mode=DR)
                nc.tensor.matmul(psum_b[:], xt_sb[:, 0:2, :],
                                 wall_sb[:, 0:2, wca:WALL_COLS],
                                 start=True, stop=False, perf_mode=DR)
                for c in range(4):
                    nc.tensor.matmul(psum_a[:, 0:K], x2t_sb[:, c, :],
                                     g_sb[:, c, :],
                                     start=False, stop=False,
                                     skip_group_check=True)
                nc.tensor.matmul(psum_a[:], xt_sb[:, 2:4, :],
                                 wall_sb[:, 2:4, 0:wca],
                                 start=False, stop=True, perf_mode=DR)
                nc.tensor.matmul(psum_b[:], xt_sb[:, 2:4, :],
                                 wall_sb[:, 2:4, wca:WALL_COLS],
                                 start=False, stop=True, perf_mode=DR)

                sq_i = sq[:, i % 2]
                nc.scalar.square(
                    sq_i[:, 0:NGA, 0:L_FAC],
                    psum_a[:, K:].rearrange("p (g t) -> p g t", t=L_FAC))
                nc.scalar.square(
                    sq_i[:, NGA:K, 0:L_FAC],
                    psum_b[:].rearrange("p (g t) -> p g t", t=L_FAC))
                nc.vector.tensor_copy(sq_i[:, :, 11], psum_a[:, 0:K])

                r1 = rpool.tile([128, K, 6], f16, tag="r1")
                nc.gpsimd.tensor_add(r1[:], sq_i[:, :, 0:6], sq_i[:, :, 6:12])

                out_sb = opool.tile([128, K], f16, tag="out")
                nc.vector.reduce_sum(out_sb[:], r1[:], axis=mybir.AxisListType.X)
                nc.sync.dma_start(out=out_dram.ap()[i * 128:(i + 1) * 128, :],
                                  in_=out_sb[:])

    nc.compile()
    return nc


_NC_CACHE = {}


def _get_nc(n_per_core=N_PER_CORE):
    if n_per_core not in _NC_CACHE:
        _NC_CACHE[n_per_core] = build_nc(n_per_core)
    return _NC_CACHE[n_per_core]


def _install_ntff_hook():
    """Provide the antenv.axon_hooks shim so trace=True can capture NTFFs."""
    import sys
    if "antenv.axon_hooks" in sys.modules:
        return
    import types
    import ctypes
    import contextlib

    so_path = "/opt/axon/libaxon_pjrt.so"
    lib = ctypes.CDLL(so_path)
    if not hasattr(lib, "axon_start_nrt_profile"):
        return
    lib.axon_start_nrt_profile.argtypes = [ctypes.POINTER(ctypes.c_int64), ctypes.c_size_t]
    lib.axon_start_nrt_profile.restype = ctypes.c_int64
    lib.axon_stop_nrt_profile.argtypes = [ctypes.c_char_p]
    lib.axon_stop_nrt_profile.restype = ctypes.c_int64

    @contextlib.contextmanager
    def _hook(output_dir, device_ids):
        import jax
        jax.devices()
        if device_ids:
            ids = (ctypes.c_int64 * len(device_ids))(*device_ids)
            rc = lib.axon_start_nrt_profile(ids, len(device_ids))
        else:
            rc = lib.axon_start_nrt_profile(None, 0)
        if rc != 0:
            raise RuntimeError(f"axon_start_nrt_profile rc={rc}")
        try:
            yield
        finally:
            n = lib.axon_stop_nrt_profile(str(output_dir).encode())
            print(f"ntff profile: {n} file(s) written to {output_dir}")

    mod = types.ModuleType("antenv.axon_hooks")
    mod.get_axon_ntff_profile_hook = lambda: _hook
    mod.set_axon_ntff_profile_hook = lambda h: None
    sys.modules["antenv.axon_hooks"] = mod


def kernel(x, MU, A, D, PI, trace=False):
    from concourse.bass_utils import run_bass_kernel_spmd
    if trace:
        try:
            _install_ntff_hook()
        except Exception as e:
            print(f"ntff hook install failed: {e}")
            trace = False

    x = np.asarray(x)
    wall, g16, cfill = host_prep(MU, A, D, PI)
    nc = _get_nc()

    in_maps = []
    for c in range(N_CORES):
        xs = np.ascontiguousarray(x[c * N_PER_CORE:(c + 1) * N_PER_CORE, :].T)
        xs = xs.astype(np.float32)
        in_maps.append({
            "xt": _tile_xt(xs, ml_dtypes.float8_e4m3),
            "x2t": _tile_xt(xs * xs, np.float16),
            "wall": wall, "g16": g16, "cfill": cfill,
        })

    res = run_bass_kernel_spmd(nc, in_maps, list(range(N_CORES)), trace=trace)
    out = np.concatenate(
        [res.results[c]["out"].astype(np.float32) for c in range(N_CORES)], axis=0)
    if trace:
        kernel.last_exec_time_ns = res.exec_time_ns
        kernel.last_results = res
    return out
